# revision 4
# baseline (speedup 1.0000x reference)
"""MoE expert FFN (swiglu) kernel for 8 trn2 NeuronCores.

Expert parallelism: 8 experts, one per core. Each core computes, for its
expert e:
    h   = x_e @ w1_e            # [2048, 2048] @ [2048, 2816]
    act = silu(h[:, :1408]) * h[:, 1408:]
    out = act @ w2_e            # [2048, 1408] @ [1408, 2048]

Tokens arrive pre-sorted by expert with equal counts (2048/expert), so
sharding is a static slice and the gather is a concat. No collectives.

Device-side layout (all bf16 compute, fp32 PSUM accumulation, fp32 out):
  mm1: out[f, t] tiles; lhsT = w1[h,f] 128x128 tiles (stationary),
       rhs = xT[h, t] (moving, N=512) -> inter is [f, t], the layout mm2
       needs, so no on-device transpose anywhere (x is transposed on host).
  swiglu pairs: psum tile f-block j (a) with f-block j+11 (b);
       act_j = silu(a) * b  via ACT(Silu) + DVE mul -> bf16 SBUF.
  mm2: out[t, h] tiles; lhsT = act[f, t] 128-col slices (stationary),
       rhs = w2[f, h] (moving, N=512). PSUM -> SBUF f32 -> DMA to out.

Weights stay resident in SBUF (bf16: 88KB + 44KB per partition); x is
streamed in 4 chunks of 512 tokens. PE-bound: ~450us of matmul per core.
"""

import os
import sys

sys.path.insert(0, "/opt/trn_rl_repo")

import numpy as np
import ml_dtypes

E = 8             # experts == cores
T_TOTAL = 16384
H = 2048
F = 1408
F2 = 2 * F        # 2816
TPC = T_TOTAL // E  # 2048 tokens per core
CHUNK = 512
NCH = TPC // CHUNK          # 4 chunks
KH = H // 128               # 16 contraction tiles for mm1
NF = F // 128               # 11 f-blocks per half (a / b)
NT = CHUNK // 128           # 4 m-tiles per chunk in mm2
NHO = H // 512              # 4 output column blocks

_CACHE = {}

# Optional knobs read by test.py (not used by the grading harness).
TRACE = os.environ.get("BASS_TRACE_KERNEL", "0") == "1"
LAST = {}


def _build():
    from concourse import bacc, tile, mybir

    bf16 = mybir.dt.bfloat16
    f32 = mybir.dt.float32
    SILU = mybir.ActivationFunctionType.Silu

    # Bacc (not plain Bass): its lowering pipeline splits multi-sem waits
    # into EventSemaphore pairs — TRN2 allows at most 1 wait per instruction.
    nc = bacc.Bacc()
    xT_d = nc.declare_dram_parameter("xT", [H, TPC], bf16, isOutput=False)
    w1_d = nc.declare_dram_parameter("w1", [H, F2], bf16, isOutput=False)
    w2_d = nc.declare_dram_parameter("w2", [F, H], bf16, isOutput=False)
    out_d = nc.declare_dram_parameter("out", [TPC, H], f32, isOutput=True)

    with tile.TileContext(nc) as tc:
        with (
            tc.tile_pool(name="w1p", bufs=1) as w1p,
            tc.tile_pool(name="w2p", bufs=1) as w2p,
            tc.tile_pool(name="xp", bufs=2) as xp,
            tc.tile_pool(name="actp", bufs=1) as actp,
            tc.tile_pool(name="tmpp", bufs=2) as tmpp,
            tc.tile_pool(name="outp", bufs=4) as outp,
            tc.tile_pool(name="psp", bufs=8, space="PSUM") as psp,
        ):
            # Resident w1, as 16x22 [128,128] column tiles so the first
            # swiglu pair only waits on 32 small DMAs, not all of w1.
            w1_t = {}
            for k in range(KH):
                for j in range(2 * NF):
                    t = w1p.tile([128, 128], bf16, tag=f"w1_{k}_{j}")
                    w1_t[(k, j)] = t
            # Resident w2: 11 tiles [128, 2048].
            w2_t = []
            for k in range(NF):
                t = w2p.tile([128, H], bf16, tag=f"w2_{k}")
                w2_t.append(t)
                nc.sync.dma_start(out=t[:], in_=w2_d[k * 128 : (k + 1) * 128, :])
            # w1 DMAs in pair-consumption order so the PE starts early.
            for j in range(NF):
                for fj in (j, NF + j):
                    for k in range(KH):
                        nc.sync.dma_start(
                            out=w1_t[(k, fj)][:],
                            in_=w1_d[k * 128 : (k + 1) * 128,
                                     fj * 128 : (fj + 1) * 128],
                        )

            for c in range(NCH):
                # Stream this chunk of tokens (columns of xT).
                x_t = []
                for k in range(KH):
                    t = xp.tile([128, CHUNK], bf16, tag=f"x_{k}")
                    x_t.append(t)
                    nc.sync.dma_start(
                        out=t[:],
                        in_=xT_d[k * 128 : (k + 1) * 128,
                                 c * CHUNK : (c + 1) * CHUNK],
                    )

                # mm1 + swiglu, one (a, b) f-block pair at a time.
                act_t = []
                for j in range(NF):
                    ps_a = psp.tile([128, CHUNK], f32, tag="ps")
                    ps_b = psp.tile([128, CHUNK], f32, tag="ps")
                    for k in range(KH):
                        nc.tensor.matmul(
                            ps_a[:],
                            w1_t[(k, j)][:],
                            x_t[k][:],
                            start=(k == 0),
                            stop=(k == KH - 1),
                        )
                    for k in range(KH):
                        nc.tensor.matmul(
                            ps_b[:],
                            w1_t[(k, NF + j)][:],
                            x_t[k][:],
                            start=(k == 0),
                            stop=(k == KH - 1),
                        )
                    tmp = tmpp.tile([128, CHUNK], f32, tag="tmp")
                    nc.scalar.activation(tmp[:], ps_a[:], SILU)
                    a = actp.tile([128, CHUNK], bf16, tag=f"act_{j}")
                    act_t.append(a)
                    nc.vector.tensor_mul(a[:], tmp[:], ps_b[:])

                # mm2: out[t, h] for this chunk.
                for m in range(NT):
                    po = [
                        psp.tile([128, 512], f32, tag="ps", name=f"po_{c}_{m}_{n}")
                        for n in range(NHO)
                    ]
                    for k in range(NF):
                        lhsT = act_t[k][:, m * 128 : (m + 1) * 128]
                        for n in range(NHO):
                            nc.tensor.matmul(
                                po[n][:],
                                lhsT,
                                w2_t[k][:, n * 512 : (n + 1) * 512],
                                start=(k == 0),
                                stop=(k == NF - 1),
                            )
                    r0 = c * CHUNK + m * 128
                    for n in range(NHO):
                        osb = outp.tile([128, 512], f32, tag="osb")
                        nc.scalar.copy(osb[:], po[n][:])
                        nc.sync.dma_start(
                            out=out_d[r0 : r0 + 128, n * 512 : (n + 1) * 512],
                            in_=osb[:],
                        )
    if not nc.is_finalized():
        nc.finalize()  # Bacc.finalize runs the lowering pipeline (sem split, alloc_regs)
    return nc


def _get_nc():
    if "nc" not in _CACHE:
        _CACHE["nc"] = _build()
    return _CACHE["nc"]


def kernel(permuted_hidden_states, num_tokens_per_expert, w1, w2):
    from concourse.bass_utils import run_bass_kernel_spmd

    x = np.asarray(permuted_hidden_states, dtype=np.float32)
    w1 = np.asarray(w1, dtype=np.float32)
    w2 = np.asarray(w2, dtype=np.float32)
    ntpe = np.asarray(num_tokens_per_expert)
    assert x.shape == (T_TOTAL, H) and w1.shape == (E, H, F2) and w2.shape == (E, F, H)
    # Reference semantics rely on the static equal split.
    assert np.all(ntpe == TPC), f"expected equal {TPC}-token splits, got {ntpe}"

    bf = ml_dtypes.bfloat16
    in_maps = []
    for e in range(E):
        xe = x[e * TPC : (e + 1) * TPC]
        in_maps.append(
            {
                "xT": np.ascontiguousarray(xe.T).astype(bf),
                "w1": np.ascontiguousarray(w1[e]).astype(bf),
                "w2": np.ascontiguousarray(w2[e]).astype(bf),
            }
        )

    nc = _get_nc()
    res = run_bass_kernel_spmd(nc, in_maps, list(range(E)), trace=TRACE)
    LAST["exec_time_ns"] = res.exec_time_ns
    LAST["mean_exec_time_ns"] = res.mean_exec_time_ns
    LAST["profile_json"] = res.profile_json
    out = np.concatenate([res.results[i]["out"] for i in range(E)], axis=0)
    return np.ascontiguousarray(out.astype(np.float32))


# revision 6
# speedup vs baseline: 1.4213x; 1.4213x over previous
"""MoE expert FFN (swiglu) kernel for 8 trn2 NeuronCores.

Expert parallelism: 8 experts, one per core. Each core computes, for its
expert e:
    h   = x_e @ w1_e            # [2048, 2048] @ [2048, 2816]
    act = silu(h[:, :1408]) * h[:, 1408:]
    out = act @ w2_e            # [2048, 1408] @ [1408, 2048]

Tokens arrive pre-sorted by expert with equal counts (2048/expert), so
sharding is a static slice and the gather is a concat. No collectives.

Device-side layout (all bf16 compute, fp32 PSUM accumulation, fp32 out):
  mm1: out[f, t] tiles; lhsT = w1[h,f] 128x128 tiles (stationary),
       rhs = xT[h, t] (moving, N=512) -> inter is [f, t], the layout mm2
       needs, so no on-device transpose anywhere (x is transposed on host).
  swiglu pairs: psum tile f-block j (a) with f-block j+11 (b);
       act_j = silu(a) * b  via ACT(Silu) + DVE mul -> bf16 SBUF.
  mm2: out[t, h] tiles; lhsT = act[f, t] 128-col slices (stationary),
       rhs = w2[f, h] (moving, N=512). PSUM -> SBUF f32 -> DMA to out.

Weights stay resident in SBUF (bf16: 88KB + 44KB per partition); x is
streamed in 4 chunks of 512 tokens. PE-bound: ~450us of matmul per core.
"""

import os
import sys

sys.path.insert(0, "/opt/trn_rl_repo")

import numpy as np
import ml_dtypes

E = 8             # experts == cores
T_TOTAL = 16384
H = 2048
F = 1408
F2 = 2 * F        # 2816
TPC = T_TOTAL // E  # 2048 tokens per core
CHUNK = 512
NCH = TPC // CHUNK          # 4 chunks
KH = H // 128               # 16 contraction tiles for mm1
NF = F // 128               # 11 f-blocks per half (a / b)
NT = CHUNK // 128           # 4 m-tiles per chunk in mm2
NHO = H // 512              # 4 output column blocks

_CACHE = {}

# Optional knobs read by test.py (not used by the grading harness).
TRACE = os.environ.get("BASS_TRACE_KERNEL", "0") == "1"
LAST = {}


def _build():
    from concourse import bacc, tile, mybir

    bf16 = mybir.dt.bfloat16
    f32 = mybir.dt.float32
    SILU = mybir.ActivationFunctionType.Silu

    # Bacc (not plain Bass): its lowering pipeline splits multi-sem waits
    # into EventSemaphore pairs — TRN2 allows at most 1 wait per instruction.
    nc = bacc.Bacc()
    xT_d = nc.declare_dram_parameter("xT", [H, TPC], bf16, isOutput=False)
    w1_d = nc.declare_dram_parameter("w1", [H, F2], bf16, isOutput=False)
    w2_d = nc.declare_dram_parameter("w2", [F, H], bf16, isOutput=False)
    out_d = nc.declare_dram_parameter("out", [TPC, H], f32, isOutput=True)

    with tile.TileContext(nc) as tc:
        with (
            tc.tile_pool(name="w1p", bufs=1) as w1p,
            tc.tile_pool(name="w2p", bufs=1) as w2p,
            tc.tile_pool(name="xp", bufs=2) as xp,
            tc.tile_pool(name="actp", bufs=1) as actp,
            tc.tile_pool(name="tmpp", bufs=2) as tmpp,
            tc.tile_pool(name="outp", bufs=4) as outp,
            tc.tile_pool(name="psp", bufs=8, space="PSUM") as psp,
        ):
            # Resident weights. Issue on the ACT HWDGE engine so weight loads
            # run concurrently with x loads on SP — a single queue serializes
            # issue (~0.5us/DMA) and stalled the PE for 227us in v1. Whole
            # [128, 2816] tiles: 22x fewer DMA instructions than column tiles.
            w1_t = []
            for k in range(KH):
                t = w1p.tile([128, F2], bf16, tag=f"w1_{k}")
                w1_t.append(t)
                nc.scalar.dma_start(out=t[:], in_=w1_d[k * 128 : (k + 1) * 128, :])
            # Resident w2: 11 tiles [128, 2048].
            w2_t = []
            for k in range(NF):
                t = w2p.tile([128, H], bf16, tag=f"w2_{k}")
                w2_t.append(t)
                nc.scalar.dma_start(out=t[:], in_=w2_d[k * 128 : (k + 1) * 128, :])

            for c in range(NCH):
                # Stream this chunk of tokens (columns of xT).
                x_t = []
                for k in range(KH):
                    t = xp.tile([128, CHUNK], bf16, tag=f"x_{k}")
                    x_t.append(t)
                    nc.sync.dma_start(
                        out=t[:],
                        in_=xT_d[k * 128 : (k + 1) * 128,
                                 c * CHUNK : (c + 1) * CHUNK],
                    )

                # mm1 + swiglu, one (a, b) f-block pair at a time.
                act_t = []
                for j in range(NF):
                    ps_a = psp.tile([128, CHUNK], f32, tag="ps")
                    ps_b = psp.tile([128, CHUNK], f32, tag="ps")
                    for k in range(KH):
                        nc.tensor.matmul(
                            ps_a[:],
                            w1_t[k][:, j * 128 : (j + 1) * 128],
                            x_t[k][:],
                            start=(k == 0),
                            stop=(k == KH - 1),
                        )
                    for k in range(KH):
                        nc.tensor.matmul(
                            ps_b[:],
                            w1_t[k][:, (NF + j) * 128 : (NF + j + 1) * 128],
                            x_t[k][:],
                            start=(k == 0),
                            stop=(k == KH - 1),
                        )
                    tmp = tmpp.tile([128, CHUNK], f32, tag="tmp")
                    nc.scalar.activation(tmp[:], ps_a[:], SILU)
                    a = actp.tile([128, CHUNK], bf16, tag=f"act_{j}")
                    act_t.append(a)
                    nc.vector.tensor_mul(a[:], tmp[:], ps_b[:])

                # mm2: out[t, h] for this chunk.
                for m in range(NT):
                    po = [
                        psp.tile([128, 512], f32, tag="ps", name=f"po_{c}_{m}_{n}")
                        for n in range(NHO)
                    ]
                    for k in range(NF):
                        lhsT = act_t[k][:, m * 128 : (m + 1) * 128]
                        for n in range(NHO):
                            nc.tensor.matmul(
                                po[n][:],
                                lhsT,
                                w2_t[k][:, n * 512 : (n + 1) * 512],
                                start=(k == 0),
                                stop=(k == NF - 1),
                            )
                    r0 = c * CHUNK + m * 128
                    for n in range(NHO):
                        osb = outp.tile([128, 512], f32, tag="osb")
                        nc.scalar.copy(osb[:], po[n][:])
                        nc.sync.dma_start(
                            out=out_d[r0 : r0 + 128, n * 512 : (n + 1) * 512],
                            in_=osb[:],
                        )
    if not nc.is_finalized():
        nc.finalize()  # Bacc.finalize runs the lowering pipeline (sem split, alloc_regs)
    return nc


def _get_nc():
    if "nc" not in _CACHE:
        _CACHE["nc"] = _build()
    return _CACHE["nc"]


def kernel(permuted_hidden_states, num_tokens_per_expert, w1, w2):
    from concourse.bass_utils import run_bass_kernel_spmd

    x = np.asarray(permuted_hidden_states, dtype=np.float32)
    w1 = np.asarray(w1, dtype=np.float32)
    w2 = np.asarray(w2, dtype=np.float32)
    ntpe = np.asarray(num_tokens_per_expert)
    assert x.shape == (T_TOTAL, H) and w1.shape == (E, H, F2) and w2.shape == (E, F, H)
    # Reference semantics rely on the static equal split.
    assert np.all(ntpe == TPC), f"expected equal {TPC}-token splits, got {ntpe}"

    bf = ml_dtypes.bfloat16
    in_maps = []
    for e in range(E):
        xe = x[e * TPC : (e + 1) * TPC]
        in_maps.append(
            {
                "xT": np.ascontiguousarray(xe.T).astype(bf),
                "w1": np.ascontiguousarray(w1[e]).astype(bf),
                "w2": np.ascontiguousarray(w2[e]).astype(bf),
            }
        )

    nc = _get_nc()
    res = run_bass_kernel_spmd(nc, in_maps, list(range(E)), trace=TRACE)
    LAST["exec_time_ns"] = res.exec_time_ns
    LAST["mean_exec_time_ns"] = res.mean_exec_time_ns
    LAST["profile_json"] = res.profile_json
    out = np.concatenate([res.results[i]["out"] for i in range(E)], axis=0)
    return np.ascontiguousarray(out.astype(np.float32))


# revision 9
# speedup vs baseline: 1.4346x; 1.0094x over previous
"""MoE expert FFN (swiglu) kernel for 8 trn2 NeuronCores.

Expert parallelism: 8 experts, one per core. Each core computes, for its
expert e:
    h   = x_e @ w1_e            # [2048, 2048] @ [2048, 2816]
    act = silu(h[:, :1408]) * h[:, 1408:]
    out = act @ w2_e            # [2048, 1408] @ [1408, 2048]

Tokens arrive pre-sorted by expert with equal counts (2048/expert), so
sharding is a static slice and the gather is a concat. No collectives.

Device-side layout (all bf16 compute, fp32 PSUM accumulation, fp32 out):
  mm1: out[f, t] tiles; lhsT = w1[h,f] 128x128 tiles (stationary),
       rhs = xT[h, t] (moving, N=512) -> inter is [f, t], the layout mm2
       needs, so no on-device transpose anywhere (x is transposed on host).
  swiglu pairs: psum tile f-block j (a) with f-block j+11 (b);
       act_j = silu(a) * b  via ACT(Silu) + DVE mul -> bf16 SBUF.
  mm2: out[t, h] tiles; lhsT = act[f, t] 128-col slices (stationary),
       rhs = w2[f, h] (moving, N=512). PSUM -> SBUF f32 -> DMA to out.

Weights stay resident in SBUF (bf16: 88KB + 44KB per partition); x is
streamed in 4 chunks of 512 tokens. PE-bound: ~450us of matmul per core.
"""

import os
import sys

sys.path.insert(0, "/opt/trn_rl_repo")

import numpy as np
import ml_dtypes

E = 8             # experts == cores
T_TOTAL = 16384
H = 2048
F = 1408
F2 = 2 * F        # 2816
TPC = T_TOTAL // E  # 2048 tokens per core
CHUNK = 512
NCH = TPC // CHUNK          # 4 chunks
KH = H // 128               # 16 contraction tiles for mm1
NF = F // 128               # 11 f-blocks per half (a / b)
NT = CHUNK // 128           # 4 m-tiles per chunk in mm2
NHO = H // 512              # 4 output column blocks

_CACHE = {}

# Optional knobs read by test.py (not used by the grading harness).
TRACE = os.environ.get("BASS_TRACE_KERNEL", "0") == "1"
LAST = {}


def _build():
    from concourse import bacc, tile, mybir

    bf16 = mybir.dt.bfloat16
    f32 = mybir.dt.float32
    SILU = mybir.ActivationFunctionType.Silu

    # Bacc (not plain Bass): its lowering pipeline splits multi-sem waits
    # into EventSemaphore pairs — TRN2 allows at most 1 wait per instruction.
    nc = bacc.Bacc()
    xT_d = nc.declare_dram_parameter("xT", [H, TPC], bf16, isOutput=False)
    w1_d = nc.declare_dram_parameter("w1", [H, F2], bf16, isOutput=False)
    w2_d = nc.declare_dram_parameter("w2", [F, H], bf16, isOutput=False)
    out_d = nc.declare_dram_parameter("out", [TPC, H], f32, isOutput=True)

    with tile.TileContext(nc) as tc:
        with (
            tc.tile_pool(name="w1p", bufs=1) as w1p,
            tc.tile_pool(name="w2p", bufs=1) as w2p,
            tc.tile_pool(name="xp", bufs=2) as xp,
            tc.tile_pool(name="actp", bufs=1) as actp,
            tc.tile_pool(name="tmpp", bufs=2) as tmpp,
            tc.tile_pool(name="outp", bufs=4) as outp,
            tc.tile_pool(name="psp", bufs=8, space="PSUM") as psp,
        ):
            # Resident weights. Issue on the ACT HWDGE engine so weight loads
            # run concurrently with x loads on SP — a single queue serializes
            # issue (~0.5us/DMA) and stalled the PE for 227us in v1. Whole
            # [128, 2816] tiles: 22x fewer DMA instructions than column tiles.
            # x chunk 0 first on SP — the first matmuls need it and SP also
            # carries half of w1 below.
            x0_t = []
            for k in range(KH):
                t = xp.tile([128, CHUNK], bf16, tag=f"x_{k}", name=f"x0_{k}")
                x0_t.append(t)
                nc.sync.dma_start(out=t[:], in_=xT_d[k * 128 : (k + 1) * 128, 0:CHUNK])

            w1_t = []
            for k in range(KH):
                t = w1p.tile([128, F2], bf16, tag=f"w1_{k}")
                w1_t.append(t)
                # Alternate engines so consecutive k-tiles stream in parallel
                # and the PE's k-accumulation stream starts sooner.
                eng = nc.scalar if k % 2 == 0 else nc.sync
                eng.dma_start(out=t[:], in_=w1_d[k * 128 : (k + 1) * 128, :])
            # Resident w2: 11 tiles [128, 2048].
            w2_t = []
            for k in range(NF):
                t = w2p.tile([128, H], bf16, tag=f"w2_{k}")
                w2_t.append(t)
                nc.scalar.dma_start(out=t[:], in_=w2_d[k * 128 : (k + 1) * 128, :])

            for c in range(NCH):
                # Stream this chunk of tokens (columns of xT); chunk 0 was
                # preloaded above.
                if c == 0:
                    x_t = x0_t
                else:
                    x_t = []
                    for k in range(KH):
                        t = xp.tile([128, CHUNK], bf16, tag=f"x_{k}", name=f"x_{c}_{k}")
                        x_t.append(t)
                        nc.sync.dma_start(
                            out=t[:],
                            in_=xT_d[k * 128 : (k + 1) * 128,
                                     c * CHUNK : (c + 1) * CHUNK],
                        )

                # mm1 + swiglu, one (a, b) f-block pair at a time.
                act_t = []
                for j in range(NF):
                    ps_a = psp.tile([128, CHUNK], f32, tag="ps")
                    ps_b = psp.tile([128, CHUNK], f32, tag="ps")
                    for k in range(KH):
                        nc.tensor.matmul(
                            ps_a[:],
                            w1_t[k][:, j * 128 : (j + 1) * 128],
                            x_t[k][:],
                            start=(k == 0),
                            stop=(k == KH - 1),
                        )
                    for k in range(KH):
                        nc.tensor.matmul(
                            ps_b[:],
                            w1_t[k][:, (NF + j) * 128 : (NF + j + 1) * 128],
                            x_t[k][:],
                            start=(k == 0),
                            stop=(k == KH - 1),
                        )
                    tmp = tmpp.tile([128, CHUNK], f32, tag="tmp")
                    nc.scalar.activation(tmp[:], ps_a[:], SILU)
                    a = actp.tile([128, CHUNK], bf16, tag=f"act_{j}")
                    act_t.append(a)
                    nc.vector.tensor_mul(a[:], tmp[:], ps_b[:])

                # mm2: out[t, h] for this chunk.
                for m in range(NT):
                    po = [
                        psp.tile([128, 512], f32, tag="ps", name=f"po_{c}_{m}_{n}")
                        for n in range(NHO)
                    ]
                    for k in range(NF):
                        lhsT = act_t[k][:, m * 128 : (m + 1) * 128]
                        for n in range(NHO):
                            nc.tensor.matmul(
                                po[n][:],
                                lhsT,
                                w2_t[k][:, n * 512 : (n + 1) * 512],
                                start=(k == 0),
                                stop=(k == NF - 1),
                            )
                    r0 = c * CHUNK + m * 128
                    for n in range(NHO):
                        osb = outp.tile([128, 512], f32, tag="osb")
                        nc.scalar.copy(osb[:], po[n][:])
                        nc.sync.dma_start(
                            out=out_d[r0 : r0 + 128, n * 512 : (n + 1) * 512],
                            in_=osb[:],
                        )
    if not nc.is_finalized():
        nc.finalize()  # Bacc.finalize runs the lowering pipeline (sem split, alloc_regs)
    return nc


def _get_nc():
    if "nc" not in _CACHE:
        _CACHE["nc"] = _build()
    return _CACHE["nc"]


def kernel(permuted_hidden_states, num_tokens_per_expert, w1, w2):
    from concourse.bass_utils import run_bass_kernel_spmd

    x = np.asarray(permuted_hidden_states, dtype=np.float32)
    w1 = np.asarray(w1, dtype=np.float32)
    w2 = np.asarray(w2, dtype=np.float32)
    ntpe = np.asarray(num_tokens_per_expert)
    assert x.shape == (T_TOTAL, H) and w1.shape == (E, H, F2) and w2.shape == (E, F, H)
    # Reference semantics rely on the static equal split.
    assert np.all(ntpe == TPC), f"expected equal {TPC}-token splits, got {ntpe}"

    bf = ml_dtypes.bfloat16
    in_maps = []
    for e in range(E):
        xe = x[e * TPC : (e + 1) * TPC]
        in_maps.append(
            {
                "xT": np.ascontiguousarray(xe.T).astype(bf),
                "w1": np.ascontiguousarray(w1[e]).astype(bf),
                "w2": np.ascontiguousarray(w2[e]).astype(bf),
            }
        )

    nc = _get_nc()
    res = run_bass_kernel_spmd(nc, in_maps, list(range(E)), trace=TRACE)
    LAST["exec_time_ns"] = res.exec_time_ns
    LAST["mean_exec_time_ns"] = res.mean_exec_time_ns
    LAST["profile_json"] = res.profile_json
    out = np.concatenate([res.results[i]["out"] for i in range(E)], axis=0)
    return np.ascontiguousarray(out.astype(np.float32))
